# revision 1
# baseline (speedup 1.0000x reference)
"""Trainium2 Bass kernel for nn_DAGLayer (gnn_message_passing).

Problem: out buffer holds L leaf columns followed by M computed nodes.
Node i gathers P=8 parent columns (each [N, C]) from the buffer, applies a
per-node dense map y = einsum('ncp,ocp->no', g, W[i]) + b[i], and appends y.

Strategy (8 NeuronCores, one SPMD program):
  - Host computes DAG levels from `parents`; nodes of one level are
    independent, so each level is one parallel "round".
  - Within a round the nodes are dealt round-robin to the 8 cores
    (node-parallel; per-node weights live only on the owning core, so the
    1 GB weight tensor is sharded 8x - the kernel is weight-DMA bound).
  - Every core keeps a replicated history buffer `hbuf` in DRAM holding all
    node outputs as [slot, c, n] fp16 blocks; after each round an AllGather
    concatenates the 8 cores' new outputs into everyone's hbuf.
  - Parent gathers are plain contiguous DMAs whose row offset is a runtime
    register loaded from a per-core offset table (single SPMD program, the
    per-core differences live entirely in input data). c-major slot layout
    puts the contraction dim on partitions directly - no transposes.
  - Per node: 16 gathered [128, 32] tiles, 32 accumulating fp16 matmuls
    (stationary weights [128k x 128o] with fast-weight-load, moving [128,32])
    into 2 PSUM tiles [128o, 32n], bias added by the psum->sbuf activation
    copy (bias is per-partition in this orientation).
  - Emission order overlaps each AllGather with the next round's weight
    DMAs and with gathers/matmuls of taps whose parents are >= 2 rounds old
    (~95% of taps), so only "fresh" taps wait on the collective.

Compute is fp16 with fp32 PSUM accumulation (the node outputs returned to
the host stay fp32). The kernel is self-contained; shapes and the schedule
are derived from the inputs at run time.
"""

import os

import numpy as np

os.environ.setdefault("NEURON_COMPILE_CACHE_URL", "/root/neuron_cache")

NCORES = 8

_BUILD_CACHE = {}


def _compute_levels(parents, L, M):
    lvl = np.zeros(L + M, np.int64)
    pare = np.asarray(parents, np.int64)
    for i in range(M):
        lvl[L + i] = 1 + lvl[pare[i]].max()
    nlev = int(lvl[L:].max()) if M else 0
    level_nodes = [np.nonzero(lvl[L:] == d)[0] for d in range(1, nlev + 1)]
    return level_nodes, lvl


def _build_bass(L, s_list, S, total_slots, old_taps):
    """old_taps[s] = list of kk in 0..15 whose parent data is >= 2 rounds old
    (may be gathered before the previous round's AllGather)."""
    import concourse.bacc as bacc
    import concourse.bass as bass
    import concourse.mybir as mybir
    import concourse.tile as tile

    f16 = mybir.dt.float16
    f32 = mybir.dt.float32
    i32 = mybir.dt.int32

    nc = bacc.Bacc(num_devices=NCORES, num_swdge_queues=4)
    # history slots are stored [c%128 (row), (c//128, n) (64 cols)] fp16 so a
    # parent gather is a single plain 2D DMA of 128 contiguous rows.
    HROWS = total_slots * 128

    wbuf = nc.dram_tensor("wbuf", [S, 128, 16, 2, 128], f16, kind="ExternalInput")
    xt = nc.dram_tensor("xt", [L * 128, 64], f16, kind="ExternalInput")
    bbuf = nc.dram_tensor("bbuf", [128, 2 * S], f32, kind="ExternalInput")
    gidx = nc.dram_tensor("gidx", [1, 8 * S], i32, kind="ExternalInput")
    yout = nc.dram_tensor("yout", [S * 128, 64], f16, kind="ExternalOutput")
    hbuf = nc.dram_tensor("hbuf", [HROWS, 64], f16, addr_space="Shared")
    agin = nc.dram_tensor("agin", [S * 128, 64], f16)
    rg = [list(range(NCORES))]

    with tile.TileContext(nc) as tc:
        with (
            tc.tile_pool(name="const", bufs=1) as constp,
            tc.tile_pool(name="w", bufs=8) as wp,
            tc.tile_pool(name="g", bufs=6) as gp,
            tc.tile_pool(name="y", bufs=8) as yp,
            tc.tile_pool(name="py", bufs=4, space="PSUM") as pyp,
        ):
            b_sb = constp.tile([128, 2 * S], f32)
            nc.sync.dma_start(b_sb[:], bbuf[:])
            gidx_sb = constp.tile([1, 8 * S], i32)
            nc.sync.dma_start(gidx_sb[:], gidx[:])
            # leaves into the shared history buffer
            nc.gpsimd.dma_start(hbuf[0 : L * 128, :], xt[:])

            # one plain 2D gather DMA per tap (128 contiguous rows of hbuf),
            # issue cost spread across the three DMA-capable sequencers.
            tap_groups = [
                (nc.sync, (0, 1, 2, 3)),
                (nc.scalar, (4, 5, 6, 7)),
            ]

            def emit_gathers(s, g_all, taps):
                tapset = set(taps)
                for eng, etaps in tap_groups:
                    use = [t for t in etaps if t in tapset]
                    if not use:
                        continue
                    lo, hi = use[0], use[-1]
                    cols = gidx_sb[0:1, 8 * s + lo : 8 * s + hi + 1]
                    _, vals = nc.values_load_multi_w_load_instructions(
                        cols,
                        engines=[eng.engine],
                        min_val=0,
                        max_val=HROWS - 128,
                        skip_runtime_bounds_check=True,
                    )
                    for t in use:
                        eng.dma_start(
                            g_all[:, t, :], hbuf[bass.ds(vals[t - lo], 128), :]
                        )

            off = 0
            pend = []  # nodes of the current round awaiting post-AG work
            pend_meta = None
            for r, s_r in enumerate(s_list):
                # phase A of round r: weight DMAs + old-tap gathers
                # (emitted BEFORE the previous round's AllGather so they only
                # depend on older AG writes via program order)
                cur = []
                for m in range(s_r):
                    s = off + m
                    w_t = wp.tile([128, 16, 2, 128], f16, tag="w")
                    nc.gpsimd.dma_start(w_t[:], wbuf[s])
                    g_all = gp.tile([128, 8, 64], f16, tag="g")
                    old = old_taps[s]
                    emit_gathers(s, g_all, old)
                    cur.append((s, w_t, g_all, old))

                # AllGather of the previous round
                if pend:
                    ps_r, poff = pend_meta
                    gbase = L + 8 * poff
                    nc.gpsimd.collective_compute(
                        "AllGather",
                        mybir.AluOpType.bypass,
                        replica_groups=rg,
                        ins=[agin[poff * 128 : (poff + ps_r) * 128, :]],
                        outs=[hbuf[gbase * 128 : (gbase + 8 * ps_r) * 128, :]],
                    )
                    pend = []

                # phase B of round r: fresh gathers, matmuls, bias, outputs
                for s, w_t, g_all, old in cur:
                    fresh = [t for t in range(8) if t not in old]
                    emit_gathers(s, g_all, fresh)
                    th_order = [(t, h) for t in list(old) + fresh for h in range(2)]
                    pys = [
                        pyp.tile([128, 32], f32, tag="py", name=f"py{oh}")
                        for oh in range(2)
                    ]
                    for i, (t, h) in enumerate(th_order):
                        for oh in range(2):
                            nc.tensor.matmul(
                                pys[oh][:],
                                w_t[:, 2 * t + h, oh, :],
                                g_all[:, t, h * 32 : (h + 1) * 32],
                                start=(i == 0),
                                stop=(i == 15),
                            )
                    y16 = yp.tile([128, 2, 32], f16, tag="y16")
                    for oh in range(2):
                        bias = b_sb[:, 2 * s + oh : 2 * s + oh + 1]
                        nc.scalar.activation(
                            y16[:, oh, :], pys[oh][:],
                            mybir.ActivationFunctionType.Identity, bias=bias,
                        )
                    nc.sync.dma_start(agin[s * 128 : (s + 1) * 128, :], y16[:])
                pend = cur
                pend_meta = (s_r, off)
                off += s_r

            # single flush of all computed node outputs
            nc.sync.dma_start(yout[:], agin[:])
    nc.compile()
    return nc


def kernel(x, W, b, parents):
    from concourse.bass_utils import run_bass_kernel_spmd

    x = np.ascontiguousarray(np.asarray(x), dtype=np.float32)
    W = np.ascontiguousarray(np.asarray(W), dtype=np.float32)
    b = np.ascontiguousarray(np.asarray(b), dtype=np.float32)
    parents = np.asarray(parents).astype(np.int64)

    N, C, L = x.shape
    M, O, C2, P = W.shape
    assert (N, C, O, C2, P) == (32, 256, 256, 256, 8), "kernel hardcodes these dims"

    level_nodes, lvl = _compute_levels(parents, L, M)
    s_list = [(len(nodes) + NCORES - 1) // NCORES for nodes in level_nodes]
    S = sum(s_list)
    total_slots = L + 8 * S

    # slot assignment: round r occupies global slots [L+8*off_r, L+8*(off_r+s_r))
    # in AllGather rank-major order; core q's m-th slot of round r holds the
    # (q + 8*m)-th node of the level.
    slot_of = np.full(L + M, -1, np.int64)
    slot_of[:L] = np.arange(L)
    node_of_coreslot = np.full((NCORES, S), -1, np.int64)
    round_of_coreslot = np.zeros(S, np.int64)
    off = 0
    for r, nodes in enumerate(level_nodes):
        s_r = s_list[r]
        round_of_coreslot[off : off + s_r] = r
        for j, node in enumerate(nodes):
            q, m = j % NCORES, j // NCORES
            slot_of[L + node] = L + 8 * off + q * s_r + m
            node_of_coreslot[q, off + m] = node
        off += s_r
    assert (slot_of >= 0).all()

    # weight relayout: [M, o, c, p] -> [M, 128(part), 16(ktile), 2(ohalf), 128(o)]
    # with k = tap*256 + c, partition = k % 128, ktile = k // 128.
    W4 = (
        W.transpose(0, 3, 2, 1)
        .reshape(M, 16, 128, 2, 128)
        .transpose(0, 2, 1, 3, 4)
        .astype(np.float16)
    )
    # leaf slot layout [c%128, (c//128, n)]: rows of 64 fp16
    xt_host = np.ascontiguousarray(
        x.transpose(2, 1, 0)
        .reshape(L, 2, 128, 32)
        .transpose(0, 2, 1, 3)
        .reshape(L * 128, 64)
        .astype(np.float16)
    )

    # old_taps[s]: tap indices whose parent was computed >= 2 rounds before
    # the slot's round (or is a leaf) on EVERY core - those gathers may be
    # emitted before the previous round's AllGather. The program structure
    # must be identical across cores, hence the intersection.
    old_taps = []
    for s in range(S):
        r = round_of_coreslot[s]
        taps = []
        for tap in range(P):
            ok = True
            for q in range(NCORES):
                node = node_of_coreslot[q, s]
                if node < 0:
                    continue
                par = parents[node][tap]
                if par >= L and lvl[par] >= r:  # parent round is lvl-1
                    ok = False
                    break
            if ok:
                taps.append(tap)
        old_taps.append(taps)

    narange = np.arange(32, dtype=np.int64)
    in_maps = []
    for q in range(NCORES):
        nodes_q = node_of_coreslot[q]
        valid = nodes_q >= 0
        Wq = np.zeros((S, 128, 16, 2, 128), np.float16)
        Wq[valid] = W4[nodes_q[valid]]
        bq = np.zeros((S, 2, 128), np.float32)
        bq[valid] = b[nodes_q[valid]].reshape(-1, 2, 128)
        # b_sb layout [128, 2S]: [o_local, (s, oh)]
        bq2 = np.ascontiguousarray(bq.transpose(2, 0, 1).reshape(128, 2 * S))
        gq = np.zeros((1, 8 * S), np.int32)
        for s in range(S):
            node = nodes_q[s]
            par = parents[node] if node >= 0 else np.zeros(P, np.int64)
            pslots = slot_of[par]
            for tap in range(P):
                gq[0, 8 * s + tap] = pslots[tap] * 128
        in_maps.append({"wbuf": Wq, "xt": xt_host, "bbuf": bq2, "gidx": gq})

    key = (L, tuple(s_list), tuple(tuple(t) for t in old_taps))
    if key not in _BUILD_CACHE:
        import time as _time

        _t0 = _time.time()
        _BUILD_CACHE[key] = _build_bass(L, s_list, S, total_slots, old_taps)
        print(f"[kernel] bass build took {_time.time() - _t0:.1f}s", flush=True)
    nc = _BUILD_CACHE[key]

    global LAST_RUN
    LAST_RUN = (nc, in_maps)

    results = run_bass_kernel_spmd(nc, in_maps, core_ids=list(range(NCORES))).results

    out = np.zeros((N, C, L + M), np.float32)
    out[:, :, :L] = x
    for q in range(NCORES):
        # yout rows are [slot, c%128] x [c//128, n]
        yq = (
            np.asarray(results[q]["yout"])
            .astype(np.float32)
            .reshape(S, 128, 2, 32)
            .transpose(0, 3, 2, 1)
            .reshape(S, 32, 256)
        )
        for s in range(S):
            node = node_of_coreslot[q, s]
            if node >= 0:
                out[:, :, L + node] = yq[s]
    return out

